# revision 29
# baseline (speedup 1.0000x reference)
"""Multi-head attention (B=4, S=2048, D=1024, H=16, dk=dv=64) on 8 TRN2 cores.

Sharding: core c = 2*b + hg handles batch b = c//2 and heads
[hg*8, hg*8+8). Each core computes a partial output
(its 8 heads' contribution through Wo); the host adds the two partials
per batch.

Per-core device pipeline (matmul inputs bf16, PSUM accumulation fp32).
The kernel is issue-ordered so the list scheduler keeps the PE gap-free
(p-state at max) and the ACT exp stream (the second-busiest engine)
starts ~14us in and never starves:

  - k-proj block 0 and q-proj(qb0,p0) are issued first; attention
    (qb0,p0) scores begin immediately after.
  - all remaining projection work (k blocks 1-3, q pairs, v chunks) is
    issued as PE filler interleaved into the attention g-loops of qb0,
    so the PE always has ready work while exp(g) -> mix(g) dependencies
    drain. v chunk t is projected just-in-time before mix needs it.
  - for qb>0, the fillers are the previous qb's Wo matmuls and the next
    qb's q projection.
  - scores^T per head pair are K=64 matmuls on partition halves
    (h0: partitions 0:64 -> PE tile (0,0); h1: 64:128 -> tile (64,0)),
    [128 keys, 512 q] fp32 in PSUM, two key chunks per [128,1024] PSUM
    tile so each ScalarE exp instruction covers 2 banks.
  - mix^T + softmax sums in one matmul: lhsT = vh_aug [128 keys, 65]
    (col 64 = mask), rhs = exp chunk half; h0/h1 accumulate into the
    two banks of one [128,1024] PSUM tile over the 16 key chunks.
  - normalize: copy the sums row (PSUM partition 64) to SBUF f32r,
    broadcast it over 64 partitions with a K=1 f32r matmul (1 cyc/row),
    reciprocal (DVE), multiply mix rows 0:64 (direct PSUM read) -> bf16;
    h1's tile is DMA-shifted to partitions 64-127 so each pair's
    mix^T is one [128, 512] tile (e on partitions).
  - out += mixT_norm.T @ Wo: dense K=128 bf16 matmuls accumulating over
    the 4 pairs; DVE evac fp32 -> DMA to HBM.

PSUM: sc ring 2x[128,1024] (scores + the normalize broadcast) = 4 banks,
mix 1x[128,1024] = 2 banks, aux ring 2x[128,512] (all projection pj +
Wo accumulators) = 2 banks.
"""

import numpy as np

B, S, D = 4, 2048, 1024
H, DK, DV = 16, 64, 64
HC = 8          # heads per core
NP = HC // 2    # head pairs per core
NCORES = 8
NC_CHUNKS = D // 128    # 8 contraction chunks over D
NKC = S // 128          # 16 key chunks
NQB = S // 512          # 4 query blocks
VW = HC * (DV + 1)      # vh storage: 65 cols per head (dv | mask)

_COMPILED = {}


def _build_nc():
    import concourse.tile as tile
    from concourse import bacc, mybir
    from contextlib import ExitStack

    F32 = mybir.dt.float32
    F32R = mybir.dt.float32r
    BF16 = mybir.dt.bfloat16
    EXP = mybir.ActivationFunctionType.Exp

    nc = bacc.Bacc("TRN2", target_bir_lowering=False, debug=False,
                   num_devices=NCORES)

    qT = nc.dram_tensor("qT", [D, S], BF16, kind="ExternalInput").ap()
    kT = nc.dram_tensor("kT", [D, S], BF16, kind="ExternalInput").ap()
    vT = nc.dram_tensor("vT", [D, S], BF16, kind="ExternalInput").ap()
    wq = nc.dram_tensor("wq", [D, HC * DK], BF16, kind="ExternalInput").ap()
    wk = nc.dram_tensor("wk", [D, HC * DK], BF16, kind="ExternalInput").ap()
    wv = nc.dram_tensor("wv", [D, HC * DV], BF16, kind="ExternalInput").ap()
    wo = nc.dram_tensor("wo", [HC * DV, D], BF16, kind="ExternalInput").ap()
    maskr = nc.dram_tensor("maskr", [128, NKC], F32, kind="ExternalInput").ap()
    out = nc.dram_tensor("out", [S, D], F32, kind="ExternalOutput").ap()

    with tile.TileContext(nc) as tc:
        with ExitStack() as ctx:
            const_pool = ctx.enter_context(tc.tile_pool(name="const", bufs=1))
            w_pool = ctx.enter_context(tc.tile_pool(name="weights", bufs=1))
            act_pool = ctx.enter_context(tc.tile_pool(name="acts", bufs=1))
            # 5 blocks of 8 stage tiles live at once (k0-k3 + q0); later
            # q blocks wrap onto long-drained slots
            st_pool = ctx.enter_context(
                tc.tile_pool(name="stage", bufs=5 * NC_CHUNKS))
            vt_pool = ctx.enter_context(tc.tile_pool(name="vstage", bufs=4))
            exp_pool = ctx.enter_context(tc.tile_pool(name="exp", bufs=6))
            norm_pool = ctx.enter_context(tc.tile_pool(name="norm", bufs=2 * NP))
            rec_pool = ctx.enter_context(tc.tile_pool(name="rec", bufs=4))
            sums_pool = ctx.enter_context(tc.tile_pool(name="sums", bufs=2))
            sh_pool = ctx.enter_context(tc.tile_pool(name="sh", bufs=4))
            osb_pool = ctx.enter_context(tc.tile_pool(name="outsb", bufs=4))
            sc_pool = ctx.enter_context(
                tc.tile_pool(name="scpsum", bufs=2, space="PSUM"))
            mix_pool = ctx.enter_context(
                tc.tile_pool(name="mxpsum", bufs=1, space="PSUM"))
            aux_pool = ctx.enter_context(
                tc.tile_pool(name="auxpsum", bufs=2, space="PSUM"))
            dram_pool = ctx.enter_context(
                tc.tile_pool(name="dscratch", bufs=4, space="DRAM"))

            mask_sb = const_pool.tile([128, NKC], F32)
            nc.sync.dma_start(mask_sb[:], maskr[:])
            ones_sb = const_pool.tile([128, HC], BF16)
            nc.vector.memset(ones_sb[:], 1.0)

            wq_sb = w_pool.tile([128, NC_CHUNKS * 512], BF16, tag="wq")
            wk_sb = w_pool.tile([128, NC_CHUNKS * 512], BF16, tag="wk")
            wv_sb = w_pool.tile([128, NC_CHUNKS * 512], BF16, tag="wv")
            wo_sb = w_pool.tile([128, NP * 1024], BF16, tag="wo")
            for c in range(NC_CHUNKS):
                nc.sync.dma_start(wk_sb[:, c * 512:(c + 1) * 512],
                                  wk[c * 128:(c + 1) * 128, :])
            for c in range(NC_CHUNKS):
                nc.sync.dma_start(wq_sb[:, c * 512:(c + 1) * 512],
                                  wq[c * 128:(c + 1) * 128, :])
                nc.sync.dma_start(wv_sb[:, c * 512:(c + 1) * 512],
                                  wv[c * 128:(c + 1) * 128, :])
            for p in range(NP):
                nc.sync.dma_start(wo_sb[:, p * 1024:(p + 1) * 1024],
                                  wo[p * 128:(p + 1) * 128, :])

            # persistent activations: per (pair, block) tiles so partial
            # projections unlock attention with fine-grained deps
            qhTb = [[act_pool.tile([128, 512], BF16, tag=f"qhT{p}_{b}",
                                   name=f"qhT{p}_{b}") for b in range(NQB)]
                    for p in range(NP)]
            khTb = [[act_pool.tile([128, 512], BF16, tag=f"khT{p}_{b}",
                                   name=f"khT{p}_{b}") for b in range(NQB)]
                    for p in range(NP)]
            vhs = [act_pool.tile([128, VW], BF16, tag=f"vh{t}", name=f"vh{t}")
                   for t in range(NKC)]

            def stage_block(src, blk):
                stg = []
                for c in range(NC_CHUNKS):
                    t = st_pool.tile([128, 512], BF16, tag="stage",
                                     name=f"stg{c}")
                    nc.sync.dma_start(
                        t[:], src[c * 128:(c + 1) * 128,
                                  blk * 512:(blk + 1) * 512])
                    stg.append(t)
                return stg

            def proj_pair(stg, wsb, dst_tile, p):
                ps = aux_pool.tile([128, 512], F32, tag="aux")
                for c in range(NC_CHUNKS):
                    nc.tensor.matmul(
                        ps[:],
                        lhsT=wsb[:, c * 512 + p * 128:
                                 c * 512 + (p + 1) * 128],
                        rhs=stg[c][:],
                        start=(c == 0), stop=(c == NC_CHUNKS - 1))
                nc.vector.tensor_copy(dst_tile[:], ps[:])

            def vproj_chunk(t):
                vt = vt_pool.tile([128, NC_CHUNKS * 128], BF16, tag="vt")
                for c in range(NC_CHUNKS):
                    nc.sync.dma_start(
                        vt[:, c * 128:(c + 1) * 128],
                        vT[c * 128:(c + 1) * 128, t * 128:(t + 1) * 128])
                ps = aux_pool.tile([128, 512], F32, tag="aux")
                for c in range(NC_CHUNKS):
                    nc.tensor.matmul(
                        ps[:],
                        lhsT=vt[:, c * 128:(c + 1) * 128],
                        rhs=wv_sb[:, c * 512:(c + 1) * 512],
                        start=(c == 0), stop=(c == NC_CHUNKS - 1))
                dst_dv = vhs[t][:, 0:VW].rearrange(
                    "p (h x) -> p h x", x=DV + 1)[:, :, 0:DV]
                src_dv = ps[:].rearrange("p (h x) -> p h x", x=DV)
                nc.vector.tensor_scalar_mul(dst_dv, src_dv,
                                            mask_sb[:, t:t + 1])
                dst_m = vhs[t][:, 0:VW].rearrange(
                    "p (h x) -> p h x", x=DV + 1)[:, :, DV:DV + 1]
                src_m = ones_sb[:, 0:HC].rearrange("p (h x) -> p h x", x=1)
                nc.vector.tensor_scalar_mul(dst_m, src_m,
                                            mask_sb[:, t:t + 1])

            # Wo for one (qb, tt, dh) triple, issued one matmul at a time
            # (4 accumulating steps + evac) so it spreads as PE filler
            wo_state = {}

            def wo_single(qb, normT, tt, dh, p):
                if p == 0:
                    wo_state[(tt, dh)] = aux_pool.tile(
                        [128, 512], F32, tag="aux", name=f"wps{tt}{dh}")
                wps = wo_state[(tt, dh)]
                nc.tensor.matmul(
                    wps[:],
                    lhsT=normT[p][:, tt * 128:(tt + 1) * 128],
                    rhs=wo_sb[:, p * 1024 + dh * 512:
                              p * 1024 + (dh + 1) * 512],
                    start=(p == 0), stop=(p == NP - 1))
                if p == NP - 1:
                    osb = osb_pool.tile([128, 512], F32, tag="osb")
                    nc.vector.tensor_copy(osb[:], wps[:])
                    nc.sync.dma_start(
                        out[qb * 512 + tt * 128:qb * 512 + (tt + 1) * 128,
                            dh * 512:(dh + 1) * 512], osb[:])

            def wo_piece(qb, normT, tt, dh):
                for p in range(NP):
                    wo_single(qb, normT, tt, dh, p)

            # ---- stage + project k block 0 / q block 0 for pair 0 ----
            kstg = [None] * NQB
            qstg = [None] * NQB
            kstg[0] = stage_block(kT, 0)
            qstg[0] = stage_block(qT, 0)
            proj_pair(kstg[0], wk_sb, khTb[0][0], 0)
            proj_pair(qstg[0], wq_sb, qhTb[0][0], 0)

            # filler thunks, one consumed at the top of each attention
            # g-iteration. The order guarantees every tile's write is
            # issued before its first read (pair p's blocks land during
            # pair p-1's loop, with p0's later k blocks interleaved
            # just ahead of the scores that need them).
            def kf(kb, p):
                return lambda: proj_pair(kstg[kb], wk_sb, khTb[p][kb], p)

            def qf(qb, p):
                return lambda: proj_pair(qstg[qb], wq_sb, qhTb[p][qb], p)

            def qb0_fillers():
                for kb in range(1, NQB):
                    kstg[kb] = stage_block(kT, kb)
                fills = [kf(1, 0), kf(0, 1), kf(2, 0), kf(0, 2),
                         kf(3, 0), kf(0, 3), qf(0, 1), kf(1, 1),
                         kf(2, 1), kf(3, 1), qf(0, 2), kf(1, 2),
                         kf(2, 2), kf(3, 2), qf(0, 3), kf(1, 3),
                         kf(2, 3), kf(3, 3)]

                def stage_q1():
                    qstg[1] = stage_block(qT, 1)
                fills.append(stage_q1)
                for p in range(NP):
                    fills.append(qf(1, p))
                return fills

            def qbn_fillers(qb, prev_normT):
                # previous block's Wo (one matmul per slot) + next
                # block's q projection
                fills = []
                for tt in range(4):
                    for dh in range(2):
                        for p in range(NP):
                            fills.append(
                                lambda tt=tt, dh=dh, p=p: wo_single(
                                    qb - 1, prev_normT, tt, dh, p))
                if qb + 1 < NQB:
                    def stage_qn():
                        qstg[qb + 1] = stage_block(qT, qb + 1)
                    fills.insert(8, stage_qn)
                    for i, p in enumerate(range(NP)):
                        fills.insert(12 + 8 * i, qf(qb + 1, p))
                return fills

            prev_normT = None
            for qb in range(NQB):
                fills = (qb0_fillers() if qb == 0
                         else qbn_fillers(qb, prev_normT))
                fills = iter(fills)

                def filler():
                    f = next(fills, None)
                    if f is not None:
                        f()

                normT = []
                for p in range(NP):
                    h0, h1 = 2 * p, 2 * p + 1
                    mix2 = mix_pool.tile([128, 1024], F32, tag="mix")
                    mixP = mix2[:, 0:512]
                    mixR = mix2[:, 512:1024]
                    l0 = slice(h0 * 65, h0 * 65 + 65)
                    l1 = slice(h1 * 65, h1 * 65 + 65)

                    def mix_g(g, exs):
                        ex0, ex1 = exs
                        for s2 in range(2):
                            kc = 2 * g + s2
                            esl = slice(s2 * 512, (s2 + 1) * 512)
                            nc.tensor.matmul(
                                mixP[0:65, :],
                                lhsT=vhs[kc][:, l0], rhs=ex0[:, esl],
                                start=(kc == 0), stop=(kc == NKC - 1))
                        for s2 in range(2):
                            kc = 2 * g + s2
                            esl = slice(s2 * 512, (s2 + 1) * 512)
                            nc.tensor.matmul(
                                mixR[0:65, :],
                                lhsT=vhs[kc][:, l1], rhs=ex1[:, esl],
                                start=(kc == 0), stop=(kc == NKC - 1))

                    # mix for group g is issued one iteration behind its
                    # exp, so the PE never sits adjacent to the ACT
                    # dependency (weights prefetch, no stall).
                    pend = None
                    for g in range(NKC // 2):
                        filler()
                        sc0 = sc_pool.tile([128, 1024], F32, tag="sc")
                        sc1 = sc_pool.tile([128, 1024], F32, tag="sc")
                        for s2 in range(2):
                            kc = 2 * g + s2
                            kb, ko = kc // 4, kc % 4
                            ksl = slice(ko * 128, (ko + 1) * 128)
                            nc.tensor.matmul(
                                sc0[:, s2 * 512:(s2 + 1) * 512],
                                lhsT=khTb[p][kb][0:64, ksl],
                                rhs=qhTb[p][qb][0:64, :],
                                start=True, stop=True)
                        for s2 in range(2):
                            kc = 2 * g + s2
                            kb, ko = kc // 4, kc % 4
                            ksl = slice(ko * 128, (ko + 1) * 128)
                            nc.tensor.matmul(
                                sc1[:, s2 * 512:(s2 + 1) * 512],
                                lhsT=khTb[p][kb][64:128, ksl],
                                rhs=qhTb[p][qb][64:128, :],
                                start=True, stop=True)
                        if qb == 0 and p == 0:
                            # project v chunks just-in-time for mix
                            vproj_chunk(2 * g)
                            vproj_chunk(2 * g + 1)
                        ex0 = exp_pool.tile([128, 1024], BF16, tag="exp")
                        ex1 = exp_pool.tile([128, 1024], BF16, tag="exp")
                        nc.scalar.activation(ex0[:], sc0[:], EXP)
                        nc.scalar.activation(ex1[:], sc1[:], EXP)
                        if pend is not None:
                            mix_g(g - 1, pend)
                        pend = (ex0, ex1)
                    mix_g(NKC // 2 - 1, pend)
                    # normalize (no PE involvement): evacuate mix rows
                    # 0:65 to SBUF (frees the PSUM tile), bounce the sums
                    # row through DRAM to broadcast it over partitions,
                    # reciprocal + scale on DVE.
                    nt = norm_pool.tile([128, 512], BF16, tag="norm")
                    normT.append(nt)
                    madd = sums_pool.tile([65, 1024], F32, tag="sums")
                    nc.vector.tensor_copy(madd[:], mix2[0:65, :])
                    dsc = dram_pool.tile([1, 1024], F32, tag="dsc")
                    nc.sync.dma_start(dsc[:], madd[64:65, :])
                    rin = rec_pool.tile([64, 1024], F32, tag="rec")
                    nc.sync.dma_start(
                        rin[:], dsc[0:1, :].to_broadcast((64, 1024)))
                    recb = rec_pool.tile([64, 1024], F32, tag="rec")
                    nc.vector.reciprocal_approx_fast(recb[:], rin[:])
                    nc.gpsimd.tensor_mul(nt[0:64, :], madd[0:64, 0:512],
                                         recb[:, 0:512])
                    sh1 = sh_pool.tile([64, 512], BF16, tag="sh1")
                    nc.gpsimd.tensor_mul(sh1[:], madd[0:64, 512:1024],
                                         recb[:, 512:1024])
                    nc.sync.dma_start(nt[64:128, :], sh1[:])

                # drain any unissued fillers for this qb
                for f in fills:
                    f()
                prev_normT = normT

            # final block's Wo
            for tt in range(4):
                for dh in range(2):
                    wo_piece(NQB - 1, prev_normT, tt, dh)

    nc.compile()
    return nc


def _get_nc():
    if "nc" not in _COMPILED:
        _COMPILED["nc"] = _build_nc()
    return _COMPILED["nc"]


def _shard_inputs(q, k, v, mask, Wq, Wk, Wv, Wo):
    """Build the per-core input maps (host-side layout prep)."""
    import ml_dtypes

    bf16 = ml_dtypes.bfloat16
    in_maps = []
    maskf = np.asarray(mask).astype(np.float32)
    q = np.asarray(q, np.float32)
    k = np.asarray(k, np.float32)
    v = np.asarray(v, np.float32)
    Wq = np.asarray(Wq, np.float32)
    Wk = np.asarray(Wk, np.float32)
    Wv = np.asarray(Wv, np.float32)
    Wo = np.asarray(Wo, np.float32)
    scale = np.float32(1.0 / np.sqrt(DK))
    for c in range(NCORES):
        b, hg = c // 2, c % 2
        hs = hg * HC
        m = {
            "qT": np.ascontiguousarray(q[b].T).astype(bf16),
            "kT": np.ascontiguousarray(k[b].T).astype(bf16),
            "vT": np.ascontiguousarray(v[b].T).astype(bf16),
            # head-major col blocks; fold 1/sqrt(dk) into Wq
            "wq": np.ascontiguousarray(
                Wq[hs:hs + HC].transpose(1, 0, 2).reshape(D, HC * DK) * scale
            ).astype(bf16),
            "wk": np.ascontiguousarray(
                Wk[hs:hs + HC].transpose(1, 0, 2).reshape(D, HC * DK)
            ).astype(bf16),
            "wv": np.ascontiguousarray(
                Wv[hs:hs + HC].transpose(1, 0, 2).reshape(D, HC * DV)
            ).astype(bf16),
            "wo": np.ascontiguousarray(Wo[hs * DV:(hs + HC) * DV]).astype(bf16),
            "maskr": np.ascontiguousarray(
                maskf[b].reshape(NKC, 128).T).astype(np.float32),
        }
        in_maps.append(m)
    return in_maps


def kernel(q, k, v, mask, Wq, Wk, Wv, Wo, _trace=False):
    from concourse.bass_utils import run_bass_kernel_spmd

    nc = _get_nc()
    in_maps = _shard_inputs(q, k, v, mask, Wq, Wk, Wv, Wo)
    res = run_bass_kernel_spmd(nc, in_maps, list(range(NCORES)),
                               trace=_trace)
    out = np.zeros((B, S, D), np.float32)
    for c in range(NCORES):
        out[c // 2] += res.results[c]["out"]
    if _trace:
        _COMPILED["last_result"] = res
    return out


# revision 30
# speedup vs baseline: 1.1837x; 1.1837x over previous
"""Multi-head attention (B=4, S=2048, D=1024, H=16, dk=dv=64) on 8 TRN2 cores.

Sharding: core c = 2*b + hg handles batch b = c//2 and heads
[hg*8, hg*8+8). Each core computes a partial output
(its 8 heads' contribution through Wo); the host adds the two partials
per batch.

Per-core device pipeline (matmul inputs bf16, PSUM accumulation fp32).
The kernel is issue-ordered so the list scheduler keeps the PE gap-free
(p-state at max) and the ACT exp stream (the second-busiest engine)
starts ~14us in and never starves:

  - k-proj block 0 and q-proj(qb0,p0) are issued first; attention
    (qb0,p0) scores begin immediately after.
  - all remaining projection work (k blocks 1-3, q pairs, v chunks) is
    issued as PE filler interleaved into the attention g-loops of qb0,
    so the PE always has ready work while exp(g) -> mix(g) dependencies
    drain. v chunk t is projected just-in-time before mix needs it.
  - for qb>0, the fillers are the previous qb's Wo matmuls and the next
    qb's q projection.
  - scores^T per head pair are K=64 matmuls on partition halves
    (h0: partitions 0:64 -> PE tile (0,0); h1: 64:128 -> tile (64,0)),
    [128 keys, 512 q] fp32 in PSUM, two key chunks per [128,1024] PSUM
    tile so each ScalarE exp instruction covers 2 banks.
  - mix^T + softmax sums in one matmul: lhsT = vh_aug [128 keys, 65]
    (col 64 = mask), rhs = exp chunk half; h0/h1 accumulate into the
    two banks of one [128,1024] PSUM tile over the 16 key chunks.
  - normalize: copy the sums row (PSUM partition 64) to SBUF f32r,
    broadcast it over 64 partitions with a K=1 f32r matmul (1 cyc/row),
    reciprocal (DVE), multiply mix rows 0:64 (direct PSUM read) -> bf16;
    h1's tile is DMA-shifted to partitions 64-127 so each pair's
    mix^T is one [128, 512] tile (e on partitions).
  - out += mixT_norm.T @ Wo: dense K=128 bf16 matmuls accumulating over
    the 4 pairs; DVE evac fp32 -> DMA to HBM.

PSUM: sc ring 2x[128,1024] (scores + the normalize broadcast) = 4 banks,
mix 1x[128,1024] = 2 banks, aux ring 2x[128,512] (all projection pj +
Wo accumulators) = 2 banks.
"""

import numpy as np

B, S, D = 4, 2048, 1024
H, DK, DV = 16, 64, 64
HC = 8          # heads per core
NP = HC // 2    # head pairs per core
NCORES = 8
NC_CHUNKS = D // 128    # 8 contraction chunks over D
NKC = S // 128          # 16 key chunks
NQB = S // 512          # 4 query blocks
VW = HC * (DV + 1)      # vh storage: 65 cols per head (dv | mask)

_COMPILED = {}


def _build_nc():
    import concourse.tile as tile
    from concourse import bacc, mybir
    from contextlib import ExitStack

    F32 = mybir.dt.float32
    F32R = mybir.dt.float32r
    BF16 = mybir.dt.bfloat16
    EXP = mybir.ActivationFunctionType.Exp

    nc = bacc.Bacc("TRN2", target_bir_lowering=False, debug=False,
                   num_devices=NCORES)

    qT = nc.dram_tensor("qT", [D, S], BF16, kind="ExternalInput").ap()
    kT = nc.dram_tensor("kT", [D, S], BF16, kind="ExternalInput").ap()
    vT = nc.dram_tensor("vT", [D, S], BF16, kind="ExternalInput").ap()
    wq = nc.dram_tensor("wq", [D, HC * DK], BF16, kind="ExternalInput").ap()
    wk = nc.dram_tensor("wk", [D, HC * DK], BF16, kind="ExternalInput").ap()
    wv = nc.dram_tensor("wv", [D, HC * DV], BF16, kind="ExternalInput").ap()
    wo = nc.dram_tensor("wo", [HC * DV, D], BF16, kind="ExternalInput").ap()
    maskr = nc.dram_tensor("maskr", [128, NKC], F32, kind="ExternalInput").ap()
    out = nc.dram_tensor("out", [S, D], F32, kind="ExternalOutput").ap()

    with tile.TileContext(nc) as tc:
        with ExitStack() as ctx:
            const_pool = ctx.enter_context(tc.tile_pool(name="const", bufs=1))
            w_pool = ctx.enter_context(tc.tile_pool(name="weights", bufs=1))
            act_pool = ctx.enter_context(tc.tile_pool(name="acts", bufs=1))
            # 5 blocks of 8 stage tiles live at once (k0-k3 + q0); later
            # q blocks wrap onto long-drained slots
            st_pool = ctx.enter_context(
                tc.tile_pool(name="stage", bufs=5 * NC_CHUNKS))
            vt_pool = ctx.enter_context(tc.tile_pool(name="vstage", bufs=4))
            exp_pool = ctx.enter_context(tc.tile_pool(name="exp", bufs=6))
            norm_pool = ctx.enter_context(tc.tile_pool(name="norm", bufs=2 * NP))
            rec_pool = ctx.enter_context(tc.tile_pool(name="rec", bufs=4))
            sums_pool = ctx.enter_context(tc.tile_pool(name="sums", bufs=2))
            sh_pool = ctx.enter_context(tc.tile_pool(name="sh", bufs=4))
            osb_pool = ctx.enter_context(tc.tile_pool(name="outsb", bufs=4))
            sc_pool = ctx.enter_context(
                tc.tile_pool(name="scpsum", bufs=2, space="PSUM"))
            mix_pool = ctx.enter_context(
                tc.tile_pool(name="mxpsum", bufs=1, space="PSUM"))
            aux_pool = ctx.enter_context(
                tc.tile_pool(name="auxpsum", bufs=2, space="PSUM"))
            dram_pool = ctx.enter_context(
                tc.tile_pool(name="dscratch", bufs=4, space="DRAM"))

            mask_sb = const_pool.tile([128, NKC], F32)
            nc.sync.dma_start(mask_sb[:], maskr[:])
            ones_sb = const_pool.tile([128, HC], BF16)
            nc.vector.memset(ones_sb[:], 1.0)

            wq_sb = w_pool.tile([128, NC_CHUNKS * 512], BF16, tag="wq")
            wk_sb = w_pool.tile([128, NC_CHUNKS * 512], BF16, tag="wk")
            wv_sb = w_pool.tile([128, NC_CHUNKS * 512], BF16, tag="wv")
            wo_sb = w_pool.tile([128, NP * 1024], BF16, tag="wo")
            for c in range(NC_CHUNKS):
                nc.sync.dma_start(wk_sb[:, c * 512:(c + 1) * 512],
                                  wk[c * 128:(c + 1) * 128, :])
            for c in range(NC_CHUNKS):
                nc.sync.dma_start(wq_sb[:, c * 512:(c + 1) * 512],
                                  wq[c * 128:(c + 1) * 128, :])
                nc.sync.dma_start(wv_sb[:, c * 512:(c + 1) * 512],
                                  wv[c * 128:(c + 1) * 128, :])
            for p in range(NP):
                nc.sync.dma_start(wo_sb[:, p * 1024:(p + 1) * 1024],
                                  wo[p * 128:(p + 1) * 128, :])

            # persistent activations: per (pair, block) tiles so partial
            # projections unlock attention with fine-grained deps
            qhTb = [[act_pool.tile([128, 512], BF16, tag=f"qhT{p}_{b}",
                                   name=f"qhT{p}_{b}") for b in range(NQB)]
                    for p in range(NP)]
            khTb = [[act_pool.tile([128, 512], BF16, tag=f"khT{p}_{b}",
                                   name=f"khT{p}_{b}") for b in range(NQB)]
                    for p in range(NP)]
            vhs = [act_pool.tile([128, VW], BF16, tag=f"vh{t}", name=f"vh{t}")
                   for t in range(NKC)]

            def stage_block(src, blk):
                stg = []
                for c in range(NC_CHUNKS):
                    t = st_pool.tile([128, 512], BF16, tag="stage",
                                     name=f"stg{c}")
                    nc.sync.dma_start(
                        t[:], src[c * 128:(c + 1) * 128,
                                  blk * 512:(blk + 1) * 512])
                    stg.append(t)
                return stg

            def proj_pair(stg, wsb, dst_tile, p):
                ps = aux_pool.tile([128, 512], F32, tag="aux")
                for c in range(NC_CHUNKS):
                    nc.tensor.matmul(
                        ps[:],
                        lhsT=wsb[:, c * 512 + p * 128:
                                 c * 512 + (p + 1) * 128],
                        rhs=stg[c][:],
                        start=(c == 0), stop=(c == NC_CHUNKS - 1))
                nc.vector.tensor_copy(dst_tile[:], ps[:])

            def vproj_chunk(t):
                vt = vt_pool.tile([128, NC_CHUNKS * 128], BF16, tag="vt")
                for c in range(NC_CHUNKS):
                    nc.sync.dma_start(
                        vt[:, c * 128:(c + 1) * 128],
                        vT[c * 128:(c + 1) * 128, t * 128:(t + 1) * 128])
                ps = aux_pool.tile([128, 512], F32, tag="aux")
                for c in range(NC_CHUNKS):
                    nc.tensor.matmul(
                        ps[:],
                        lhsT=vt[:, c * 128:(c + 1) * 128],
                        rhs=wv_sb[:, c * 512:(c + 1) * 512],
                        start=(c == 0), stop=(c == NC_CHUNKS - 1))
                dst_dv = vhs[t][:, 0:VW].rearrange(
                    "p (h x) -> p h x", x=DV + 1)[:, :, 0:DV]
                src_dv = ps[:].rearrange("p (h x) -> p h x", x=DV)
                nc.vector.tensor_scalar_mul(dst_dv, src_dv,
                                            mask_sb[:, t:t + 1])
                dst_m = vhs[t][:, 0:VW].rearrange(
                    "p (h x) -> p h x", x=DV + 1)[:, :, DV:DV + 1]
                src_m = ones_sb[:, 0:HC].rearrange("p (h x) -> p h x", x=1)
                nc.vector.tensor_scalar_mul(dst_m, src_m,
                                            mask_sb[:, t:t + 1])

            # Wo for one (qb, tt, dh) triple, issued one matmul at a time
            # (4 accumulating steps + evac) so it spreads as PE filler
            wo_state = {}

            def wo_single(qb, normT, tt, dh, p):
                if p == 0:
                    wo_state[(tt, dh)] = aux_pool.tile(
                        [128, 512], F32, tag="aux", name=f"wps{tt}{dh}")
                wps = wo_state[(tt, dh)]
                nc.tensor.matmul(
                    wps[:],
                    lhsT=normT[p][:, tt * 128:(tt + 1) * 128],
                    rhs=wo_sb[:, p * 1024 + dh * 512:
                              p * 1024 + (dh + 1) * 512],
                    start=(p == 0), stop=(p == NP - 1))
                if p == NP - 1:
                    osb = osb_pool.tile([128, 512], F32, tag="osb")
                    nc.vector.tensor_copy(osb[:], wps[:])
                    nc.sync.dma_start(
                        out[qb * 512 + tt * 128:qb * 512 + (tt + 1) * 128,
                            dh * 512:(dh + 1) * 512], osb[:])

            def wo_piece(qb, normT, tt, dh):
                for p in range(NP):
                    wo_single(qb, normT, tt, dh, p)

            # ---- stage + project k block 0 / q block 0 for pair 0 ----
            kstg = [None] * NQB
            qstg = [None] * NQB
            kstg[0] = stage_block(kT, 0)
            qstg[0] = stage_block(qT, 0)
            proj_pair(kstg[0], wk_sb, khTb[0][0], 0)
            proj_pair(qstg[0], wq_sb, qhTb[0][0], 0)

            # filler thunks, one consumed at the top of each attention
            # g-iteration. The order guarantees every tile's write is
            # issued before its first read (pair p's blocks land during
            # pair p-1's loop, with p0's later k blocks interleaved
            # just ahead of the scores that need them).
            def kf(kb, p):
                return lambda: proj_pair(kstg[kb], wk_sb, khTb[p][kb], p)

            def qf(qb, p):
                return lambda: proj_pair(qstg[qb], wq_sb, qhTb[p][qb], p)

            def qb0_fillers():
                for kb in range(1, NQB):
                    kstg[kb] = stage_block(kT, kb)
                fills = [kf(1, 0), kf(0, 1), kf(2, 0), kf(0, 2),
                         kf(3, 0), kf(0, 3), qf(0, 1), kf(1, 1),
                         kf(2, 1), kf(3, 1), qf(0, 2), kf(1, 2),
                         kf(2, 2), kf(3, 2), qf(0, 3), kf(1, 3),
                         kf(2, 3), kf(3, 3)]

                def stage_q1():
                    qstg[1] = stage_block(qT, 1)
                fills.append(stage_q1)
                for p in range(NP):
                    fills.append(qf(1, p))
                return fills

            def qbn_fillers(qb, prev_normT):
                # previous block's Wo (one matmul per slot) + next
                # block's q projection
                fills = []
                for tt in range(4):
                    for dh in range(2):
                        for p in range(NP):
                            fills.append(
                                lambda tt=tt, dh=dh, p=p: wo_single(
                                    qb - 1, prev_normT, tt, dh, p))
                if qb + 1 < NQB:
                    def stage_qn():
                        qstg[qb + 1] = stage_block(qT, qb + 1)
                    fills.insert(8, stage_qn)
                    for i, p in enumerate(range(NP)):
                        fills.insert(12 + 8 * i, qf(qb + 1, p))
                return fills

            prev_normT = None
            for qb in range(NQB):
                fills = (qb0_fillers() if qb == 0
                         else qbn_fillers(qb, prev_normT))
                fills = iter(fills)

                def filler():
                    f = next(fills, None)
                    if f is not None:
                        f()

                normT = []
                for p in range(NP):
                    h0, h1 = 2 * p, 2 * p + 1
                    mix2 = mix_pool.tile([128, 1024], F32, tag="mix")
                    mixP = mix2[:, 0:512]
                    mixR = mix2[:, 512:1024]
                    l0 = slice(h0 * 65, h0 * 65 + 65)
                    l1 = slice(h1 * 65, h1 * 65 + 65)

                    def mix_g(g, exs):
                        ex0, ex1 = exs
                        for s2 in range(2):
                            kc = 2 * g + s2
                            esl = slice(s2 * 512, (s2 + 1) * 512)
                            nc.tensor.matmul(
                                mixP[0:65, :],
                                lhsT=vhs[kc][:, l0], rhs=ex0[:, esl],
                                start=(kc == 0), stop=(kc == NKC - 1))
                        for s2 in range(2):
                            kc = 2 * g + s2
                            esl = slice(s2 * 512, (s2 + 1) * 512)
                            nc.tensor.matmul(
                                mixR[0:65, :],
                                lhsT=vhs[kc][:, l1], rhs=ex1[:, esl],
                                start=(kc == 0), stop=(kc == NKC - 1))

                    # mix for group g is issued one iteration behind its
                    # exp, so the PE never sits adjacent to the ACT
                    # dependency (weights prefetch, no stall).
                    pend = None
                    for g in range(NKC // 2):
                        filler()
                        sc0 = sc_pool.tile([128, 1024], F32, tag="sc")
                        sc1 = sc_pool.tile([128, 1024], F32, tag="sc")
                        for s2 in range(2):
                            kc = 2 * g + s2
                            kb, ko = kc // 4, kc % 4
                            ksl = slice(ko * 128, (ko + 1) * 128)
                            nc.tensor.matmul(
                                sc0[:, s2 * 512:(s2 + 1) * 512],
                                lhsT=khTb[p][kb][0:64, ksl],
                                rhs=qhTb[p][qb][0:64, :],
                                start=True, stop=True)
                        for s2 in range(2):
                            kc = 2 * g + s2
                            kb, ko = kc // 4, kc % 4
                            ksl = slice(ko * 128, (ko + 1) * 128)
                            nc.tensor.matmul(
                                sc1[:, s2 * 512:(s2 + 1) * 512],
                                lhsT=khTb[p][kb][64:128, ksl],
                                rhs=qhTb[p][qb][64:128, :],
                                start=True, stop=True)
                        if qb == 0 and p == 0:
                            # project v chunks just-in-time for mix
                            vproj_chunk(2 * g)
                            vproj_chunk(2 * g + 1)
                        ex0 = exp_pool.tile([128, 1024], BF16, tag="exp")
                        ex1 = exp_pool.tile([128, 1024], BF16, tag="exp")
                        nc.scalar.activation(ex0[:], sc0[:], EXP)
                        nc.scalar.activation(ex1[:], sc1[:], EXP)
                        if pend is not None:
                            mix_g(g - 1, pend)
                        pend = (ex0, ex1)
                    mix_g(NKC // 2 - 1, pend)
                    # normalize (no PE involvement): evacuate mix rows
                    # 0:65 to SBUF (frees the PSUM tile), bounce the sums
                    # row through DRAM to broadcast it over partitions,
                    # reciprocal + scale on DVE.
                    nt = norm_pool.tile([128, 512], BF16, tag="norm")
                    normT.append(nt)
                    madd = sums_pool.tile([65, 1024], F32, tag="sums")
                    nc.vector.tensor_copy(madd[:], mix2[0:65, :])
                    dsc = dram_pool.tile([1, 1024], F32, tag="dsc")
                    nc.sync.dma_start(dsc[:], madd[64:65, :])
                    rin = rec_pool.tile([64, 1024], F32, tag="rec")
                    nc.sync.dma_start(
                        rin[:], dsc[0:1, :].to_broadcast((64, 1024)))
                    recb = rec_pool.tile([64, 1024], F32, tag="rec")
                    nc.vector.reciprocal_approx_fast(recb[:], rin[:])
                    nc.vector.tensor_mul(nt[0:64, :], madd[0:64, 0:512],
                                         recb[:, 0:512])
                    sh1 = sh_pool.tile([64, 512], BF16, tag="sh1")
                    nc.vector.tensor_mul(sh1[:], madd[0:64, 512:1024],
                                         recb[:, 512:1024])
                    nc.sync.dma_start(nt[64:128, :], sh1[:])

                # drain any unissued fillers for this qb
                for f in fills:
                    f()
                prev_normT = normT

            # final block's Wo
            for tt in range(4):
                for dh in range(2):
                    wo_piece(NQB - 1, prev_normT, tt, dh)

    nc.compile()
    return nc


def _get_nc():
    if "nc" not in _COMPILED:
        _COMPILED["nc"] = _build_nc()
    return _COMPILED["nc"]


def _shard_inputs(q, k, v, mask, Wq, Wk, Wv, Wo):
    """Build the per-core input maps (host-side layout prep)."""
    import ml_dtypes

    bf16 = ml_dtypes.bfloat16
    in_maps = []
    maskf = np.asarray(mask).astype(np.float32)
    q = np.asarray(q, np.float32)
    k = np.asarray(k, np.float32)
    v = np.asarray(v, np.float32)
    Wq = np.asarray(Wq, np.float32)
    Wk = np.asarray(Wk, np.float32)
    Wv = np.asarray(Wv, np.float32)
    Wo = np.asarray(Wo, np.float32)
    scale = np.float32(1.0 / np.sqrt(DK))
    for c in range(NCORES):
        b, hg = c // 2, c % 2
        hs = hg * HC
        m = {
            "qT": np.ascontiguousarray(q[b].T).astype(bf16),
            "kT": np.ascontiguousarray(k[b].T).astype(bf16),
            "vT": np.ascontiguousarray(v[b].T).astype(bf16),
            # head-major col blocks; fold 1/sqrt(dk) into Wq
            "wq": np.ascontiguousarray(
                Wq[hs:hs + HC].transpose(1, 0, 2).reshape(D, HC * DK) * scale
            ).astype(bf16),
            "wk": np.ascontiguousarray(
                Wk[hs:hs + HC].transpose(1, 0, 2).reshape(D, HC * DK)
            ).astype(bf16),
            "wv": np.ascontiguousarray(
                Wv[hs:hs + HC].transpose(1, 0, 2).reshape(D, HC * DV)
            ).astype(bf16),
            "wo": np.ascontiguousarray(Wo[hs * DV:(hs + HC) * DV]).astype(bf16),
            "maskr": np.ascontiguousarray(
                maskf[b].reshape(NKC, 128).T).astype(np.float32),
        }
        in_maps.append(m)
    return in_maps


def kernel(q, k, v, mask, Wq, Wk, Wv, Wo, _trace=False):
    from concourse.bass_utils import run_bass_kernel_spmd

    nc = _get_nc()
    in_maps = _shard_inputs(q, k, v, mask, Wq, Wk, Wv, Wo)
    res = run_bass_kernel_spmd(nc, in_maps, list(range(NCORES)),
                               trace=_trace)
    out = np.zeros((B, S, D), np.float32)
    for c in range(NCORES):
        out[c // 2] += res.results[c]["out"]
    if _trace:
        _COMPILED["last_result"] = res
    return out
